# revision 1
# baseline (speedup 1.0000x reference)
"""AutoRec scoring kernel for 8x Trainium2 NeuronCores (Bass/Tile).

Computation (see problem reference):
    agg   = segment_sum(dat[:,None] * v[cols], rows, m)    # COO spmm (m,d)
    h     = sigmoid(agg + mu)                              # (m,d)
    score = sum(h[i] * w[j], -1) + b[j]                    # (P,)

Sharding: edges sharded by row range (8 equal ranges of 6250 rows), pairs
sharded by i range (same ranges) -> no cross-core communication at all.
v/w/mu/b replicated per core.

Per core:
  phase 1: for each 128-row block, gather v[col] rows (dma_gather, bf16,
           col-sorted for HBM locality, 4 SWDGE queues), scatter-add edges
           to rows with a one-hot-times-dat matmul into PSUM
           (lhsT A[e,r] = dat(e) if row(e)==r), add mu via a K=1
           ones-matmul, sigmoid (ACT) -> h block kept resident in SBUF.
  phase 2 (fused): pairs are grouped by (j-half, i-block) and j-sorted;
           gather w[j] rows (dma_gather); expand h rows to pair slots with
           a one-hot fp8 matmul (psum = M_T.T @ h_block); elementwise
           multiply + reduce (split DVE/ACT) -> scores.

Host does index preprocessing only (sort/bucket/pad + building the one-hot
matrices); all FLOPs and all gathers run on device.
"""

import os
import sys

import numpy as np

for _p in ("/opt/trn_rl_repo",):
    if os.path.isdir(_p) and _p not in sys.path:
        sys.path.insert(0, _p)

import ml_dtypes  # noqa: E402

import concourse.bacc as bacc  # noqa: E402
import concourse.mybir as mybir  # noqa: E402
import concourse.tile as tile  # noqa: E402
from concourse.bass_utils import run_bass_kernel_spmd  # noqa: E402

BF16 = ml_dtypes.bfloat16
FP8 = ml_dtypes.float8_e4m3

NCORES = 8
M = 50000
N = 50000
D = 256
RPC = M // NCORES          # rows per core: 6250
BLOCKS = (RPC + 127) // 128  # 49
HALF = 32768               # int16 index limit; v/w split into two halves
G = 4096                   # phase-2 w-gather call size (slots)

_BUILD_CACHE = {}
LAST_RESULTS = None


def _build_program(CAPL, CAPH, CL, CH, WW):
    """Build the SPMD bass program (same instructions on all cores).

    CAPL/CAPH: phase-1 gather-chunk capacities per (block, col-half).
    CL/CH: phase-2 pair-tile capacities per block for j-lo / j-hi segments.
    """
    CAP = CAPL + CAPH
    LT = sum(CL)               # lo-segment tiles
    HT = sum(CH)
    TT = LT + HT               # total pair tiles
    TS = TT * 128
    f32 = mybir.dt.float32
    bf16 = mybir.dt.bfloat16
    fp8 = mybir.dt.float8e4
    i16 = mybir.dt.int16

    # block of each pair tile, in slot order (lo segment then hi segment)
    tile_block = []
    for blk in range(BLOCKS):
        tile_block += [blk] * CL[blk]
    for blk in range(BLOCKS):
        tile_block += [blk] * CH[blk]

    nc = bacc.Bacc("TRN2", target_bir_lowering=False, debug=False,
                   num_devices=NCORES, num_swdge_queues=4)

    # ---- DRAM tensors ----
    v_bf = nc.dram_tensor("v_bf", [N, D], bf16, kind="ExternalInput")
    a_t = nc.dram_tensor("a_t", [BLOCKS, 128, CAP * 128], bf16,
                         kind="ExternalInput")
    gi_lo = nc.dram_tensor("gi_lo", [BLOCKS, 128, CAPL * 8], i16,
                           kind="ExternalInput")
    gi_hi = nc.dram_tensor("gi_hi", [BLOCKS, 128, CAPH * 8], i16,
                           kind="ExternalInput")
    mu_bf = nc.dram_tensor("mu_bf", [1, D], bf16, kind="ExternalInput")
    w_bf = nc.dram_tensor("w_bf", [N, WW], bf16, kind="ExternalInput")
    wj_t = nc.dram_tensor("wj_t", [128, TS // 16], i16, kind="ExternalInput")
    m_t = nc.dram_tensor("m_t", [128, TT, 128], fp8, kind="ExternalInput")
    scores = nc.dram_tensor("scores", [128, TT], f32, kind="ExternalOutput")

    with tile.TileContext(nc) as tc:
        import contextlib
        with contextlib.ExitStack() as ctx:
            const_p = ctx.enter_context(tc.tile_pool(name="const", bufs=1))
            a_p = ctx.enter_context(tc.tile_pool(name="a", bufs=2))
            gi_p = ctx.enter_context(tc.tile_pool(name="gi", bufs=2))
            x_p = ctx.enter_context(tc.tile_pool(name="x", bufs=2))
            ps_p = ctx.enter_context(
                tc.tile_pool(name="ps", bufs=2, space="PSUM"))
            pe_p = ctx.enter_context(
                tc.tile_pool(name="pe", bufs=4, space="PSUM"))
            wt_p = ctx.enter_context(tc.tile_pool(name="wt", bufs=2))
            mt_p = ctx.enter_context(tc.tile_pool(name="mt", bufs=2))
            pi_p = ctx.enter_context(tc.tile_pool(name="pi", bufs=2))
            pr_p = ctx.enter_context(tc.tile_pool(name="pr", bufs=4))

            ones_k1 = const_p.tile([1, 128], bf16)
            nc.vector.memset(ones_k1[:], 1.0)
            mu_sb = const_p.tile([1, D], bf16)
            nc.sync.dma_start(mu_sb[:], mu_bf[:, :])
            sc_sb = const_p.tile([128, TT], f32)
            # all h blocks stay resident in SBUF
            h_all = const_p.tile([128, BLOCKS, WW], bf16)

            DUP = int(os.environ.get("K_DUP", "1"))
            qrr = [0]

            def nextq():
                qrr[0] += 1
                return qrr[0] % 4

            for _rep in range(DUP):
                # ---------------- phase 1 ----------------
                for blk in range(BLOCKS):
                    at = a_p.tile([128, CAP * 128], bf16)
                    nc.sync.dma_start(at[:], a_t[blk, :, :])
                    gl = gi_p.tile([128, CAPL * 8], i16, tag="gil")
                    nc.sync.dma_start(gl[:], gi_lo[blk, :, :])
                    gh = gi_p.tile([128, CAPH * 8], i16, tag="gih")
                    nc.sync.dma_start(gh[:], gi_hi[blk, :, :])

                    xt = x_p.tile([128, CAP, D], bf16)
                    nc.gpsimd.dma_gather(
                        xt[:, 0:CAPL, :], v_bf[0:HALF, :], gl[:],
                        num_idxs=CAPL * 128, num_idxs_reg=CAPL * 128,
                        elem_size=D, single_packet=False,
                        queue_num=nextq())
                    nc.gpsimd.dma_gather(
                        xt[:, CAPL:CAP, :], v_bf[HALF:N, :], gh[:],
                        num_idxs=CAPH * 128, num_idxs_reg=CAPH * 128,
                        elem_size=D, single_packet=False,
                        queue_num=nextq())

                    ps = ps_p.tile([128, D], f32)
                    for c in range(CAP):
                        nc.tensor.matmul(
                            ps[:], lhsT=at[:, c * 128:(c + 1) * 128],
                            rhs=xt[:, c, :], start=(c == 0), stop=False)
                    nc.tensor.matmul(ps[:], lhsT=ones_k1[:], rhs=mu_sb[:],
                                     start=False, stop=True)
                    nc.scalar.activation(
                        h_all[:, blk, 0:D], ps[:],
                        mybir.ActivationFunctionType.Sigmoid)
                    if WW > D:
                        nc.vector.memset(h_all[:, blk, D:WW], 0.0)
                        nc.vector.memset(h_all[:, blk, D:D + 1], 1.0)

                # ---------------- phase 2 (fused) ----------------
                calls = []
                for off in range(0, LT * 128, G):
                    calls.append((off, min(G, LT * 128 - off), 0))
                for off in range(LT * 128, TS, G):
                    calls.append((off, min(G, TS - off), 1))

                for (off, sl, hseg) in calls:
                    nt = sl // 128
                    t0_ = off // 128
                    wit = pi_p.tile([128, sl // 16], i16, tag="wit")
                    nc.sync.dma_start(
                        wit[:], wj_t[:, off // 16:(off + sl) // 16])
                    wtile = wt_p.tile([128, nt, WW], bf16)
                    wsrc = w_bf[0:HALF, :] if hseg == 0 else w_bf[HALF:N, :]
                    nc.gpsimd.dma_gather(
                        wtile[:], wsrc, wit[:], num_idxs=sl, num_idxs_reg=sl,
                        elem_size=WW, single_packet=False,
                        queue_num=nextq())
                    mtile = mt_p.tile([128, nt, 128], fp8)
                    nc.sync.dma_start(mtile[:], m_t[:, t0_:t0_ + nt, :])

                    for t in range(nt):
                        gt = t0_ + t
                        blk = tile_block[gt]
                        pex = pe_p.tile([128, WW], f32)
                        nc.tensor.matmul(
                            pex[:], lhsT=mtile[:, t, :],
                            rhs=h_all[:, blk, :], start=True, stop=True)
                        prod = pr_p.tile([128, WW], bf16)
                        nc.vector.tensor_tensor(
                            out=prod[:], in0=pex[:], in1=wtile[:, t, :],
                            op=mybir.AluOpType.mult)
                        if t % 2 == 0:
                            trash = pr_p.tile([128, WW], bf16, tag="trash")
                            nc.scalar.activation(
                                trash[:], prod[:],
                                mybir.ActivationFunctionType.Identity,
                                accum_out=sc_sb[:, gt:gt + 1])
                        else:
                            nc.vector.tensor_reduce(
                                out=sc_sb[:, gt:gt + 1], in_=prod[:],
                                axis=mybir.AxisListType.X,
                                op=mybir.AluOpType.add)

            nc.sync.dma_start(scores[:, :], sc_sb[:])

    nc.compile()
    return nc


def kernel(idx, dat, m, n, i, j, v, mu, w, b):
    global LAST_RESULTS
    idx = np.asarray(idx)
    dat = np.asarray(dat, np.float32)
    i = np.asarray(i).astype(np.int64)
    j = np.asarray(j).astype(np.int64)
    v = np.asarray(v, np.float32)
    mu_np = np.asarray(mu, np.float32).reshape(1, D)
    w_np = np.asarray(w, np.float32)
    b_np = np.asarray(b, np.float32).reshape(-1)
    rows = idx[0].astype(np.int64)
    cols = idx[1].astype(np.int64)
    NNZ = rows.shape[0]
    P = i.shape[0]
    assert int(m) == M and int(n) == N
    assert v.shape == (N, D) and w_np.shape == (N, D)

    use_b = bool(np.any(b_np))
    WW = 384 if use_b else 256

    # ---------------- phase 1 host prep ----------------
    core_e = rows // RPC
    lrow = rows - core_e * RPC
    blk = lrow >> 7
    r_in_blk = (lrow & 127).astype(np.int64)
    half = (cols >= HALF).astype(np.int64)

    gkey = (core_e * BLOCKS + blk) * 2 + half
    order = np.lexsort((cols, gkey))
    gsorted = gkey[order]
    ngroups = NCORES * BLOCKS * 2
    counts = np.bincount(gsorted, minlength=ngroups)
    cnt_lo = counts[0::2]
    cnt_hi = counts[1::2]
    CAPL = max(1, int(np.ceil(cnt_lo.max() / 128)))
    CAPH = max(1, int(np.ceil(cnt_hi.max() / 128)))
    CAP = CAPL + CAPH
    gstart = np.zeros(ngroups + 1, np.int64)
    gstart[1:] = np.cumsum(counts)
    pos_in_group = np.arange(NNZ) - gstart[gsorted]
    eslot = pos_in_group + (gsorted % 2) * (CAPL * 128)
    g2 = gsorted // 2
    e_core = g2 // BLOCKS
    e_blk = g2 % BLOCKS
    echunk = eslot >> 7
    e_in_chunk = eslot & 127

    A = np.zeros((NCORES, BLOCKS, 128, CAP, 128), BF16)
    A[e_core, e_blk, e_in_chunk, echunk, r_in_blk[order]] = \
        dat[order].astype(BF16)

    gi = np.zeros((NCORES, BLOCKS, CAP * 128), np.int16)
    colv = (cols[order] - half[order] * HALF).astype(np.int16)
    gi[e_core, e_blk, eslot] = colv
    gil = gi[:, :, :CAPL * 128].reshape(NCORES, BLOCKS, CAPL * 8, 16)
    gih = gi[:, :, CAPL * 128:].reshape(NCORES, BLOCKS, CAPH * 8, 16)
    gi_lo = np.tile(gil.swapaxes(2, 3), (1, 1, 8, 1))
    gi_hi = np.tile(gih.swapaxes(2, 3), (1, 1, 8, 1))

    # ---------------- phase 2 host prep ----------------
    p_core = i // RPC
    il = (i - p_core * RPC).astype(np.int64)
    pblk = il >> 7
    r_il = (il & 127).astype(np.int64)
    jhalf = (j >= HALF).astype(np.int64)
    # group key: (core, jhalf, block); j-sorted inside each group
    pkey = (p_core * 2 + jhalf) * BLOCKS + pblk
    porder = np.lexsort((j, pkey))
    ksort = pkey[porder]
    npg = NCORES * 2 * BLOCKS
    pcounts = np.bincount(ksort, minlength=npg)
    pc3 = pcounts.reshape(NCORES, 2, BLOCKS)
    # per-block tile capacities (max over cores), separate lo/hi
    CL = [max(1, int(np.ceil(pc3[:, 0, bb].max() / 128)))
          for bb in range(BLOCKS)]
    CH = [max(1, int(np.ceil(pc3[:, 1, bb].max() / 128)))
          for bb in range(BLOCKS)]
    LT = sum(CL)
    HT = sum(CH)
    TT = LT + HT
    TS = TT * 128

    # slot base per (jhalf, block) group: lo groups by block, then hi
    base = np.zeros((2, BLOCKS), np.int64)
    acc = 0
    for bb in range(BLOCKS):
        base[0, bb] = acc
        acc += CL[bb] * 128
    for bb in range(BLOCKS):
        base[1, bb] = acc
        acc += CH[bb] * 128
    assert acc == TS

    pstart = np.zeros(npg + 1, np.int64)
    pstart[1:] = np.cumsum(pcounts)
    pos2 = np.arange(P) - pstart[ksort]
    k_half = (ksort // BLOCKS) % 2
    k_blk = ksort % BLOCKS
    k_core = ksort // (2 * BLOCKS)
    slot = base[k_half, k_blk] + pos2
    wj = np.zeros((NCORES, TS), np.int16)
    wj[k_core, slot] = (j[porder] - k_half * HALF).astype(np.int16)
    slot_of_pair = np.empty(P, np.int64)
    slot_of_pair[porder] = k_core * TS + slot

    wj_t = np.tile(wj.reshape(NCORES, TS // 16, 16).swapaxes(1, 2),
                   (1, 8, 1))

    # one-hot expansion matrices M_T[t, r, p] (fp8, exact 1.0)
    MT = np.zeros((NCORES, 128, TT, 128), FP8)
    one8 = np.float32(1.0).astype(FP8)
    MT[k_core, r_il[porder], slot >> 7, slot & 127] = one8

    # ---------------- build inputs ----------------
    v_bf = np.ascontiguousarray(v.astype(BF16))
    if use_b:
        w_aug = np.zeros((N, WW), np.float32)
        w_aug[:, :D] = w_np
        w_aug[:, D] = b_np
        w_bf = np.ascontiguousarray(w_aug.astype(BF16))
    else:
        w_bf = np.ascontiguousarray(w_np.astype(BF16))
    mu_bf = np.ascontiguousarray(mu_np.astype(BF16))

    key = (CAPL, CAPH, tuple(CL), tuple(CH), WW)
    if _BUILD_CACHE.get("key") != key:
        _BUILD_CACHE.clear()
        _BUILD_CACHE["key"] = key
        _BUILD_CACHE["nc"] = _build_program(CAPL, CAPH, CL, CH, WW)
    nc = _BUILD_CACHE["nc"]

    in_maps = []
    for c in range(NCORES):
        in_maps.append({
            "v_bf": v_bf,
            "a_t": np.ascontiguousarray(
                A[c].reshape(BLOCKS, 128, CAP * 128)),
            "gi_lo": gi_lo[c],
            "gi_hi": gi_hi[c],
            "mu_bf": mu_bf,
            "w_bf": w_bf,
            "wj_t": wj_t[c],
            "m_t": MT[c],
        })

    res = run_bass_kernel_spmd(
        nc, in_maps, core_ids=list(range(NCORES)),
        trace=bool(int(os.environ.get("KERNEL_TRACE", "0"))))
    LAST_RESULTS = res

    if os.environ.get("KERNEL_BENCH", "0") == "1":
        _benchmark(nc, in_maps)

    flat = np.concatenate(
        [res.results[c]["scores"].T.reshape(-1) for c in range(NCORES)])
    return flat[slot_of_pair].astype(np.float32)


def _benchmark(nc, in_maps, iters=10):
    import time
    run, _ = _make_bench(nc, in_maps)
    for _ in range(2):
        run()
    times = []
    for _ in range(iters):
        t0 = time.perf_counter()
        run()
        times.append(time.perf_counter() - t0)
    times = np.array(times)
    print(f"exec wall: min {times.min()*1e6:.0f} us  "
          f"median {np.median(times)*1e6:.0f} us  "
          f"mean {times.mean()*1e6:.0f} us")
    print(f"HW exec time: {times.min()*1e9:.0f} ns")
    return times


def _make_bench(nc, in_maps):
    """Build a timed executor: inputs pre-placed on device (mirrors
    bass2jax.run_bass_via_pjrt's multi-core path)."""
    import jax
    from jax.sharding import Mesh, NamedSharding, PartitionSpec

    from concourse import bass2jax
    from concourse.bass2jax import _bass_exec_p, install_neuronx_cc_hook

    install_neuronx_cc_hook()
    n_cores = NCORES
    part_name = (nc.partition_id_tensor.name
                 if nc.partition_id_tensor else None)
    in_names = []
    out_names = []
    out_avals = []
    zero_outs = []
    for alloc in nc.m.functions[0].allocations:
        if not isinstance(alloc, mybir.MemoryLocationSet):
            continue
        name = alloc.memorylocations[0].name
        if alloc.kind == "ExternalInput":
            if name != part_name:
                in_names.append(name)
        elif alloc.kind == "ExternalOutput":
            out_names.append(name)
            shape = tuple(alloc.tensor_shape)
            dtype = mybir.dt.np(alloc.dtype)
            out_avals.append(jax.core.ShapedArray(shape, dtype))
            zero_outs.append(np.zeros(shape, dtype))
    n_params = len(in_names)
    n_outs = len(out_avals)
    all_names = in_names + out_names
    if part_name is not None:
        all_names = all_names + [part_name]

    nrep = int(os.environ.get("K_NREP", "1"))

    def _body(*args):
        ins = list(args[:n_params])
        outs_all = []
        for r in range(nrep):
            operands = ins + list(
                args[n_params + r * n_outs:n_params + (r + 1) * n_outs])
            if part_name is not None:
                operands.append(bass2jax.partition_id_tensor())
            outs = _bass_exec_p.bind(
                *operands,
                out_avals=tuple(out_avals),
                in_names=tuple(all_names),
                out_names=tuple(out_names),
                lowering_input_output_aliases=(),
                sim_require_finite=True,
                sim_require_nnan=True,
                nc=nc,
            )
            outs_all.extend(outs)
        return tuple(outs_all)

    devices = jax.devices()[:n_cores]
    mesh = Mesh(np.asarray(devices), ("core",))
    shard_map = bass2jax.shard_map
    n_zeros = nrep * n_outs
    sharded = jax.jit(
        shard_map(_body, mesh=mesh,
                  in_specs=(PartitionSpec("core"),) * (n_params + n_zeros),
                  out_specs=(PartitionSpec("core"),) * n_zeros,
                  check_rep=False),
        donate_argnums=tuple(range(n_params, n_params + n_zeros)),
        keep_unused=True)

    sh = NamedSharding(mesh, PartitionSpec("core"))
    dev_in = [
        jax.device_put(
            np.concatenate([np.asarray(in_maps[c][nm]) for c in
                            range(n_cores)], axis=0), sh)
        for nm in in_names]
    concat_zeros = [np.zeros((n_cores * z.shape[0], *z.shape[1:]), z.dtype)
                    for z in zero_outs] * nrep

    def run():
        outs = sharded(*dev_in, *concat_zeros)
        jax.block_until_ready(outs)
        return outs

    return run, nrep



# revision 11
# speedup vs baseline: 1.0334x; 1.0334x over previous
"""AutoRec scoring kernel for 8x Trainium2 NeuronCores (Bass/Tile).

Computation (see problem reference):
    agg   = segment_sum(dat[:,None] * v[cols], rows, m)    # COO spmm (m,d)
    h     = sigmoid(agg + mu)                              # (m,d)
    score = sum(h[i] * w[j], -1) + b[j]                    # (P,)

Sharding: edges sharded by row range (8 equal ranges of 6250 rows), pairs
sharded by i range (same ranges) -> no cross-core communication.
v/w/mu replicated per core.

Per core:
  phase 1: for each 128-row block, gather v[col] rows (dma_gather, bf16,
           col-sorted for HBM locality), build the dat one-hot scatter
           matrix ON DEVICE from compact (row, dat) tables via
           iota==row tensor_scalar ops (DVE/GPSIMD), matmul-accumulate
           into PSUM, add mu (K=1 ones matmul), sigmoid -> h block,
           DMA h block to a DRAM scratch tile.
  phase 2: pairs sharded by i range, grouped by j-half (int16 gather
           indices), j-sorted. Gather h[i] rows from the DRAM scratch
           and w[j] rows from HBM; one fused tensor_tensor_reduce per
           128-pair tile (mult + row-reduce) -> scores (bf16 out).

Host does index preprocessing only (sort/bucket/pad); all FLOPs and all
gathers run on device. No fat one-hot matrices are shipped from host.
"""

import os
import sys

import numpy as np

for _p in ("/opt/trn_rl_repo",):
    if os.path.isdir(_p) and _p not in sys.path:
        sys.path.insert(0, _p)

import ml_dtypes  # noqa: E402

import concourse.bacc as bacc  # noqa: E402
import concourse.mybir as mybir  # noqa: E402
import concourse.tile as tile  # noqa: E402
from concourse.bass_utils import run_bass_kernel_spmd  # noqa: E402

BF16 = ml_dtypes.bfloat16

NCORES = 8
M = 50000
N = 50000
D = 256
RPC = M // NCORES            # rows per core: 6250
BLOCKS = (RPC + 127) // 128  # 49
NHPAD = BLOCKS * 128         # padded h rows per core: 6272
HALF = 32768                 # int16 index limit; v/w split into two halves
GT = 32                      # phase-2 tiles per gather call (4096 slots)

_BUILD_CACHE = {}
LAST_RESULTS = None


def _build_program(CAPL, CAPH, TLO, THI, WW):
    """Build the SPMD bass program (same instructions on all cores).

    CAPL/CAPH: phase-1 gather-chunk capacities per (block, col-half).
    TLO/THI: phase-2 pair-tile capacities for j-lo / j-hi segments.
    """
    CAP = CAPL + CAPH
    TT = TLO + THI
    TS = TT * 128
    f32 = mybir.dt.float32
    bf16 = mybir.dt.bfloat16
    i16 = mybir.dt.int16
    i32 = mybir.dt.int32

    nc = bacc.Bacc("TRN2", target_bir_lowering=False, debug=False,
                   num_devices=NCORES, num_swdge_queues=1,
                   dynamic_dma_scratch_size=65536)

    # ---- DRAM tensors ----
    v_bf = nc.dram_tensor("v_bf", [N, D], bf16, kind="ExternalInput")
    w_bf = nc.dram_tensor("w_bf", [N, WW], bf16, kind="ExternalInput")
    mu_bf = nc.dram_tensor("mu_bf", [1, D], bf16, kind="ExternalInput")
    er_t = nc.dram_tensor("er_t", [BLOCKS, 128, CAP], f32,
                          kind="ExternalInput")
    ed_t = nc.dram_tensor("ed_t", [BLOCKS, 128, CAP], f32,
                          kind="ExternalInput")
    gi_lo = nc.dram_tensor("gi_lo", [BLOCKS, 128, CAPL * 8], i16,
                           kind="ExternalInput")
    gi_hi = nc.dram_tensor("gi_hi", [BLOCKS, 128, CAPH * 8], i16,
                           kind="ExternalInput")
    hi_t = nc.dram_tensor("hi_t", [128, TS // 16], i16, kind="ExternalInput")
    wi_t = nc.dram_tensor("wi_t", [128, TS // 16], i16, kind="ExternalInput")
    scores = nc.dram_tensor("scores", [128, TT], f32, kind="ExternalOutput")

    with tile.TileContext(nc) as tc:
        import contextlib
        with contextlib.ExitStack() as ctx:
            const_p = ctx.enter_context(tc.tile_pool(name="const", bufs=1))
            hd_p = ctx.enter_context(
                tc.tile_pool(name="hd", bufs=1, space="DRAM"))
            et_p = ctx.enter_context(tc.tile_pool(name="et", bufs=2))
            gi_p = ctx.enter_context(tc.tile_pool(name="gi", bufs=2))
            a_p = ctx.enter_context(tc.tile_pool(name="a", bufs=2))
            x_p = ctx.enter_context(tc.tile_pool(name="x", bufs=2))
            h_p = ctx.enter_context(tc.tile_pool(name="h", bufs=2))
            ps_p = ctx.enter_context(
                tc.tile_pool(name="ps", bufs=2, space="PSUM"))
            pi_p = ctx.enter_context(tc.tile_pool(name="pi", bufs=2))
            ht_p = ctx.enter_context(tc.tile_pool(name="ht", bufs=2))
            wt_p = ctx.enter_context(tc.tile_pool(name="wt", bufs=2))
            tr_p = ctx.enter_context(tc.tile_pool(name="tr", bufs=4))

            iota_r = const_p.tile([128, 128], bf16)
            nc.gpsimd.iota(iota_r[:], pattern=[[1, 128]],
                           channel_multiplier=0,
                           allow_small_or_imprecise_dtypes=True)
            ones_k1 = const_p.tile([1, 128], bf16)
            nc.vector.memset(ones_k1[:], 1.0)
            mu_sb = const_p.tile([1, D], bf16)
            nc.sync.dma_start(mu_sb[:], mu_bf[:, :])
            sc_sb = const_p.tile([128, TT], f32)

            h_d = hd_p.tile([NHPAD, WW], bf16)

            def nextq():
                return 0

            # ---------------- phase 1 ----------------
            for blk in range(BLOCKS):
                er_sb = et_p.tile([128, CAP], f32, tag="er")
                nc.sync.dma_start(er_sb[:], er_t[blk, :, :])
                ed_sb = et_p.tile([128, CAP], f32, tag="ed")
                nc.sync.dma_start(ed_sb[:], ed_t[blk, :, :])
                gl = gi_p.tile([128, CAPL * 8], i16, tag="gil")
                nc.sync.dma_start(gl[:], gi_lo[blk, :, :])
                gh = gi_p.tile([128, CAPH * 8], i16, tag="gih")
                nc.sync.dma_start(gh[:], gi_hi[blk, :, :])

                xt = x_p.tile([128, CAP, D], bf16)
                nc.gpsimd.dma_gather(
                    xt[:, 0:CAPL, :], v_bf[0:HALF, :], gl[:],
                    num_idxs=CAPL * 128, num_idxs_reg=CAPL * 128,
                    elem_size=D, single_packet=False, queue_num=nextq())
                nc.gpsimd.dma_gather(
                    xt[:, CAPL:CAP, :], v_bf[HALF:N, :], gh[:],
                    num_idxs=CAPH * 128, num_idxs_reg=CAPH * 128,
                    elem_size=D, single_packet=False, queue_num=nextq())

                a_sb = a_p.tile([128, CAP, 128], bf16)
                for c in range(CAP):
                    eng = nc.vector
                    eng.tensor_scalar(
                        out=a_sb[:, c, :], in0=iota_r[:],
                        scalar1=er_sb[:, c:c + 1],
                        scalar2=ed_sb[:, c:c + 1],
                        op0=mybir.AluOpType.is_equal,
                        op1=mybir.AluOpType.mult)

                ps = ps_p.tile([128, D], f32)
                for c in range(CAP):
                    nc.tensor.matmul(
                        ps[:], lhsT=a_sb[:, c, :], rhs=xt[:, c, :],
                        start=(c == 0), stop=False)
                nc.tensor.matmul(ps[:], lhsT=ones_k1[:], rhs=mu_sb[:],
                                 start=False, stop=True)
                h_sb = h_p.tile([128, WW], bf16)
                nc.scalar.activation(
                    h_sb[:, 0:D], ps[:],
                    mybir.ActivationFunctionType.Sigmoid)
                if WW > D:
                    nc.vector.memset(h_sb[:, D:WW], 0.0)
                    nc.vector.memset(h_sb[:, D:D + 1], 1.0)
                nc.sync.dma_start(h_d[blk * 128:(blk + 1) * 128, :], h_sb[:])

            # ---------------- phase 2 ----------------
            calls = []
            for off in range(0, TLO * 128, GT * 128):
                calls.append((off, min(GT * 128, TLO * 128 - off), 0))
            for off in range(TLO * 128, TS, GT * 128):
                calls.append((off, min(GT * 128, TS - off), 1))

            for (off, sl, hseg) in calls:
                nt = sl // 128
                t0_ = off // 128
                hit = pi_p.tile([128, sl // 16], i16, tag="hit")
                nc.sync.dma_start(
                    hit[:], hi_t[:, off // 16:(off + sl) // 16])
                wit = pi_p.tile([128, sl // 16], i16, tag="wit")
                nc.sync.dma_start(
                    wit[:], wi_t[:, off // 16:(off + sl) // 16])

                ht = ht_p.tile([128, nt, WW], bf16)
                nc.gpsimd.dma_gather(
                    ht[:], h_d[:, :], hit[:], num_idxs=sl, num_idxs_reg=sl,
                    elem_size=WW, single_packet=False, queue_num=nextq())
                wt = wt_p.tile([128, nt, WW], bf16)
                wsrc = w_bf[0:HALF, :] if hseg == 0 else w_bf[HALF:N, :]
                nc.gpsimd.dma_gather(
                    wt[:], wsrc, wit[:], num_idxs=sl, num_idxs_reg=sl,
                    elem_size=WW, single_packet=False, queue_num=nextq())

                for t in range(nt):
                    gt = t0_ + t
                    prod = tr_p.tile([128, WW], bf16)
                    nc.vector.tensor_tensor(
                        out=prod[:], in0=ht[:, t, :], in1=wt[:, t, :],
                        op=mybir.AluOpType.mult)
                    if t % 2 == 0:
                        trash = tr_p.tile([128, WW], bf16, tag="trash")
                        nc.scalar.activation(
                            trash[:], prod[:],
                            mybir.ActivationFunctionType.Identity,
                            accum_out=sc_sb[:, gt:gt + 1])
                    else:
                        nc.vector.tensor_reduce(
                            out=sc_sb[:, gt:gt + 1], in_=prod[:],
                            axis=mybir.AxisListType.X,
                            op=mybir.AluOpType.add)

            nc.sync.dma_start(scores[:, :], sc_sb[:])

    nc.compile()
    return nc


def _idx_table(idx2d):
    """[G, n] int16 -> [G, 128, n//16] wrapped-16/replicated-8 format."""
    G, n = idx2d.shape
    arr = idx2d.reshape(G, n // 16, 16).swapaxes(1, 2)  # [G, 16, n//16]
    return np.ascontiguousarray(np.tile(arr, (1, 8, 1)))


def kernel(idx, dat, m, n, i, j, v, mu, w, b):
    global LAST_RESULTS
    idx = np.asarray(idx)
    dat = np.asarray(dat, np.float32)
    i = np.asarray(i).astype(np.int64)
    j = np.asarray(j).astype(np.int64)
    v = np.asarray(v, np.float32)
    mu_np = np.asarray(mu, np.float32).reshape(1, D)
    w_np = np.asarray(w, np.float32)
    b_np = np.asarray(b, np.float32).reshape(-1)
    rows = idx[0].astype(np.int64)
    cols = idx[1].astype(np.int64)
    NNZ = rows.shape[0]
    P = i.shape[0]
    assert int(m) == M and int(n) == N
    assert v.shape == (N, D) and w_np.shape == (N, D)

    use_b = bool(np.any(b_np))
    WW = 384 if use_b else 256

    # ---------------- phase 1 host prep ----------------
    core_e = rows // RPC
    lrow = rows - core_e * RPC
    blk = lrow >> 7
    r_in_blk = (lrow & 127).astype(np.int64)
    half = (cols >= HALF).astype(np.int64)

    gkey = (core_e * BLOCKS + blk) * 2 + half
    order = np.lexsort((cols, gkey))
    gsorted = gkey[order]
    ngroups = NCORES * BLOCKS * 2
    counts = np.bincount(gsorted, minlength=ngroups)
    cnt_lo = counts[0::2]
    cnt_hi = counts[1::2]
    CAPL = max(1, int(np.ceil(cnt_lo.max() / 128)))
    CAPH = max(1, int(np.ceil(cnt_hi.max() / 128)))
    CAP = CAPL + CAPH
    gstart = np.zeros(ngroups + 1, np.int64)
    gstart[1:] = np.cumsum(counts)
    pos_in_group = np.arange(NNZ) - gstart[gsorted]
    eslot = pos_in_group + (gsorted % 2) * (CAPL * 128)
    g2 = gsorted // 2
    e_core = g2 // BLOCKS
    e_blk = g2 % BLOCKS
    echunk = eslot >> 7
    e_in_chunk = eslot & 127

    er_np = np.zeros((NCORES, BLOCKS, 128, CAP), np.float32)
    ed_np = np.zeros((NCORES, BLOCKS, 128, CAP), np.float32)
    er_np[e_core, e_blk, e_in_chunk, echunk] = r_in_blk[order]
    ed_np[e_core, e_blk, e_in_chunk, echunk] = dat[order]

    gi = np.zeros((NCORES, BLOCKS, CAP * 128), np.int16)
    gi[e_core, e_blk, eslot] = (cols[order] - half[order] * HALF).astype(
        np.int16)
    gi_lo = _idx_table(gi[:, :, :CAPL * 128].reshape(-1, CAPL * 128)).reshape(
        NCORES, BLOCKS, 128, CAPL * 8)
    gi_hi = _idx_table(gi[:, :, CAPL * 128:].reshape(-1, CAPH * 128)).reshape(
        NCORES, BLOCKS, 128, CAPH * 8)

    # ---------------- phase 2 host prep ----------------
    p_core = i // RPC
    il = (i - p_core * RPC).astype(np.int64)
    jhalf = (j >= HALF).astype(np.int64)
    pkey = p_core * 2 + jhalf
    porder = np.lexsort((j, pkey))
    ksort = pkey[porder]
    pcounts = np.bincount(ksort, minlength=NCORES * 2).reshape(NCORES, 2)
    TLO = max(1, int(np.ceil(pcounts[:, 0].max() / 128)))
    THI = max(1, int(np.ceil(pcounts[:, 1].max() / 128)))
    TT = TLO + THI
    TS = TT * 128

    pstart = np.zeros(NCORES * 2 + 1, np.int64)
    pstart[1:] = np.cumsum(pcounts.reshape(-1))
    pos2 = np.arange(P) - pstart[ksort]
    k_half = ksort % 2
    k_core = ksort // 2
    slot = k_half * (TLO * 128) + pos2

    hi_np = np.zeros((NCORES, TS), np.int16)
    hi_np[k_core, slot] = il[porder].astype(np.int16)
    wi_np = np.zeros((NCORES, TS), np.int16)
    wi_np[k_core, slot] = (j[porder] - k_half * HALF).astype(np.int16)
    slot_of_pair = np.empty(P, np.int64)
    slot_of_pair[porder] = k_core * TS + slot

    hi_t = _idx_table(hi_np)
    wi_t = _idx_table(wi_np)

    # ---------------- build inputs ----------------
    v_bf = np.ascontiguousarray(v.astype(BF16))
    if use_b:
        w_aug = np.zeros((N, WW), np.float32)
        w_aug[:, :D] = w_np
        w_aug[:, D] = b_np
        w_bf = np.ascontiguousarray(w_aug.astype(BF16))
    else:
        w_bf = np.ascontiguousarray(w_np.astype(BF16))
    mu_bf = np.ascontiguousarray(mu_np.astype(BF16))

    key = (CAPL, CAPH, TLO, THI, WW)
    if _BUILD_CACHE.get("key") != key:
        _BUILD_CACHE.clear()
        _BUILD_CACHE["key"] = key
        _BUILD_CACHE["nc"] = _build_program(CAPL, CAPH, TLO, THI, WW)
    nc = _BUILD_CACHE["nc"]

    in_maps = []
    for c in range(NCORES):
        in_maps.append({
            "v_bf": v_bf,
            "w_bf": w_bf,
            "mu_bf": mu_bf,
            "er_t": er_np[c],
            "ed_t": ed_np[c],
            "gi_lo": gi_lo[c],
            "gi_hi": gi_hi[c],
            "hi_t": hi_t[c],
            "wi_t": wi_t[c],
        })

    res = run_bass_kernel_spmd(
        nc, in_maps, core_ids=list(range(NCORES)),
        trace=bool(int(os.environ.get("KERNEL_TRACE", "0"))))
    LAST_RESULTS = res

    if os.environ.get("KERNEL_BENCH", "0") == "1":
        _benchmark(nc, in_maps)

    flat = np.concatenate(
        [np.asarray(res.results[c]["scores"], np.float32).T.reshape(-1)
         for c in range(NCORES)])
    return flat[slot_of_pair].astype(np.float32)


def _benchmark(nc, in_maps, iters=10):
    import time

    def timeit(run, n):
        for _ in range(2):
            run()
        ts = []
        for _ in range(n):
            t0 = time.perf_counter()
            run()
            ts.append(time.perf_counter() - t0)
        return np.array(ts)

    run1, _ = _make_bench(nc, in_maps, nrep=1)
    times = timeit(run1, iters)
    print(f"exec wall: min {times.min()*1e6:.0f} us  "
          f"median {np.median(times)*1e6:.0f} us  "
          f"mean {times.mean()*1e6:.0f} us")
    print(f"HW exec time: {times.min()*1e9:.0f} ns")
    nrep = int(os.environ.get("K_NREP", "0"))
    if nrep > 1:
        runN, _ = _make_bench(nc, in_maps, nrep=nrep)
        tN = timeit(runN, iters)
        body = (tN.min() - times.min()) / (nrep - 1)
        print(f"nrep={nrep} wall: min {tN.min()*1e6:.0f} us  "
              f"median {np.median(tN)*1e6:.0f} us")
        print(f"body estimate: {body*1e6:.1f} us per exec")
    return times


def _make_bench(nc, in_maps, nrep=None):
    """Build a timed executor: inputs pre-placed on device (mirrors
    bass2jax.run_bass_via_pjrt's multi-core path)."""
    import jax
    from jax.sharding import Mesh, NamedSharding, PartitionSpec

    from concourse import bass2jax
    from concourse.bass2jax import _bass_exec_p, install_neuronx_cc_hook

    install_neuronx_cc_hook()
    n_cores = NCORES
    part_name = (nc.partition_id_tensor.name
                 if nc.partition_id_tensor else None)
    in_names = []
    out_names = []
    out_avals = []
    zero_outs = []
    for alloc in nc.m.functions[0].allocations:
        if not isinstance(alloc, mybir.MemoryLocationSet):
            continue
        name = alloc.memorylocations[0].name
        if alloc.kind == "ExternalInput":
            if name != part_name:
                in_names.append(name)
        elif alloc.kind == "ExternalOutput":
            out_names.append(name)
            shape = tuple(alloc.tensor_shape)
            dtype = mybir.dt.np(alloc.dtype)
            out_avals.append(jax.core.ShapedArray(shape, dtype))
            zero_outs.append(np.zeros(shape, dtype))
    n_params = len(in_names)
    n_outs = len(out_avals)
    all_names = in_names + out_names
    if part_name is not None:
        all_names = all_names + [part_name]

    if nrep is None:
        nrep = 1

    def _body(*args):
        ins = list(args[:n_params])
        outs_all = []
        for r in range(nrep):
            operands = ins + list(
                args[n_params + r * n_outs:n_params + (r + 1) * n_outs])
            if part_name is not None:
                operands.append(bass2jax.partition_id_tensor())
            outs = _bass_exec_p.bind(
                *operands,
                out_avals=tuple(out_avals),
                in_names=tuple(all_names),
                out_names=tuple(out_names),
                lowering_input_output_aliases=(),
                sim_require_finite=True,
                sim_require_nnan=True,
                nc=nc,
            )
            outs_all.extend(outs)
        return tuple(outs_all)

    devices = jax.devices()[:n_cores]
    mesh = Mesh(np.asarray(devices), ("core",))
    shard_map = bass2jax.shard_map
    n_zeros = nrep * n_outs
    sharded = jax.jit(
        shard_map(_body, mesh=mesh,
                  in_specs=(PartitionSpec("core"),) * (n_params + n_zeros),
                  out_specs=(PartitionSpec("core"),) * n_zeros,
                  check_rep=False),
        donate_argnums=tuple(range(n_params, n_params + n_zeros)),
        keep_unused=True)

    sh = NamedSharding(mesh, PartitionSpec("core"))
    dev_in = [
        jax.device_put(
            np.concatenate([np.asarray(in_maps[c][nm]) for c in
                            range(n_cores)], axis=0), sh)
        for nm in in_names]
    concat_zeros = [np.zeros((n_cores * z.shape[0], *z.shape[1:]), z.dtype)
                    for z in zero_outs] * nrep

    def run():
        outs = sharded(*dev_in, *concat_zeros)
        jax.block_until_ready(outs)
        return outs

    return run, nrep


# revision 14
# speedup vs baseline: 1.6329x; 1.5801x over previous
"""AutoRec scoring kernel for 8x Trainium2 NeuronCores (Bass/Tile).

Computation (see problem reference):
    agg   = segment_sum(dat[:,None] * v[cols], rows, m)    # COO spmm (m,d)
    h     = sigmoid(agg + mu)                              # (m,d)
    score = sum(h[i] * w[j], -1) + b[j]                    # (P,)

Sharding: edges sharded by row range (8 equal ranges of 6250 rows), pairs
sharded by i range (same ranges) -> no cross-core communication.
v/w/mu replicated per core.

Per core:
  phase 1: for each 128-row block, gather v[col] rows (dma_gather, bf16,
           col-sorted for HBM locality), build the dat one-hot scatter
           matrix ON DEVICE from compact (row, dat) tables via
           iota==row tensor_scalar ops (DVE/GPSIMD), matmul-accumulate
           into PSUM, add mu (K=1 ones matmul), sigmoid -> h block,
           DMA h block to a DRAM scratch tile.
  phase 2: pairs sharded by i range, grouped by j-half (int16 gather
           indices), j-sorted. Gather h[i] rows from the DRAM scratch
           and w[j] rows from HBM; one fused tensor_tensor_reduce per
           128-pair tile (mult + row-reduce) -> scores (bf16 out).

Host does index preprocessing only (sort/bucket/pad); all FLOPs and all
gathers run on device. No fat one-hot matrices are shipped from host.
"""

import os
import sys

import numpy as np

for _p in ("/opt/trn_rl_repo",):
    if os.path.isdir(_p) and _p not in sys.path:
        sys.path.insert(0, _p)

import ml_dtypes  # noqa: E402

import concourse.bacc as bacc  # noqa: E402
import concourse.mybir as mybir  # noqa: E402
import concourse.tile as tile  # noqa: E402
from concourse.bass_utils import run_bass_kernel_spmd  # noqa: E402

BF16 = ml_dtypes.bfloat16

NCORES = 8
M = 50000
N = 50000
D = 256
RPC = M // NCORES            # rows per core: 6250
BLOCKS = (RPC + 127) // 128  # 49
NHPAD = BLOCKS * 128         # padded h rows per core: 6272
HALF = 32768                 # int16 index limit; v/w split into two halves
GT = 32                      # phase-2 tiles per gather call (4096 slots)

_BUILD_CACHE = {}
LAST_RESULTS = None


def _build_program(CAPL, CAPH, TLO, THI, WW):
    """Build the SPMD bass program (same instructions on all cores).

    CAPL/CAPH: phase-1 gather-chunk capacities per (block, col-half).
    TLO/THI: phase-2 pair-tile capacities for j-lo / j-hi segments.
    """
    CAP = CAPL + CAPH
    TT = TLO + THI
    TS = TT * 128
    f32 = mybir.dt.float32
    bf16 = mybir.dt.bfloat16
    i16 = mybir.dt.int16
    i32 = mybir.dt.int32

    nc = bacc.Bacc("TRN2", target_bir_lowering=False, debug=False,
                   num_devices=NCORES, num_swdge_queues=4,
                   dynamic_dma_scratch_size=65536)

    # ---- DRAM tensors ----
    v_bf = nc.dram_tensor("v_bf", [N, D], bf16, kind="ExternalInput")
    w_bf = nc.dram_tensor("w_bf", [N, WW], bf16, kind="ExternalInput")
    mu_bf = nc.dram_tensor("mu_bf", [1, D], bf16, kind="ExternalInput")
    er_t = nc.dram_tensor("er_t", [BLOCKS, 128, CAP], f32,
                          kind="ExternalInput")
    ed_t = nc.dram_tensor("ed_t", [BLOCKS, 128, CAP], f32,
                          kind="ExternalInput")
    gi_lo = nc.dram_tensor("gi_lo", [BLOCKS, 128, CAPL * 8], i16,
                           kind="ExternalInput")
    gi_hi = nc.dram_tensor("gi_hi", [BLOCKS, 128, CAPH * 8], i16,
                           kind="ExternalInput")
    ri_t = nc.dram_tensor("ri_t", [128, TS // 16], i16, kind="ExternalInput")
    wi_t = nc.dram_tensor("wi_t", [128, TS // 16], i16, kind="ExternalInput")
    id_t = nc.dram_tensor("id_t", [128, 128], bf16, kind="ExternalInput")
    scores = nc.dram_tensor("scores", [128, TT], f32, kind="ExternalOutput")

    with tile.TileContext(nc) as tc:
        import contextlib
        with contextlib.ExitStack() as ctx:
            const_p = ctx.enter_context(tc.tile_pool(name="const", bufs=1))
            et_p = ctx.enter_context(tc.tile_pool(name="et", bufs=2))
            gi_p = ctx.enter_context(tc.tile_pool(name="gi", bufs=2))
            a_p = ctx.enter_context(tc.tile_pool(name="a", bufs=2))
            x_p = ctx.enter_context(tc.tile_pool(name="x", bufs=2))
            h_p = ctx.enter_context(tc.tile_pool(name="h", bufs=2))
            ps_p = ctx.enter_context(
                tc.tile_pool(name="ps", bufs=2, space="PSUM"))
            pe_p = ctx.enter_context(
                tc.tile_pool(name="pe", bufs=4, space="PSUM"))
            mt_p = ctx.enter_context(tc.tile_pool(name="mt", bufs=2))
            pi_p = ctx.enter_context(tc.tile_pool(name="pi", bufs=2))
            ht_p = ctx.enter_context(tc.tile_pool(name="ht", bufs=2))
            wt_p = ctx.enter_context(tc.tile_pool(name="wt", bufs=2))
            tr_p = ctx.enter_context(tc.tile_pool(name="tr", bufs=4))

            iota_r = const_p.tile([128, 128], bf16)
            nc.gpsimd.iota(iota_r[:], pattern=[[1, 128]],
                           channel_multiplier=0,
                           allow_small_or_imprecise_dtypes=True)
            ones_k1 = const_p.tile([1, 128], bf16)
            nc.vector.memset(ones_k1[:], 1.0)
            mu_sb = const_p.tile([1, D], bf16)
            nc.sync.dma_start(mu_sb[:], mu_bf[:, :])
            sc_sb = const_p.tile([128, TT], f32)
            h_all = const_p.tile([128, BLOCKS, WW], bf16)

            qctr = [0]

            def nextq():
                k = qctr[0]
                qctr[0] += 1
                if queues is not None and k < len(queues):
                    return queues[k]
                return 0

            # ---------------- phase 1 ----------------
            for blk in range(BLOCKS):
                er_sb = et_p.tile([128, CAP], f32, tag="er")
                nc.sync.dma_start(er_sb[:], er_t[blk, :, :])
                ed_sb = et_p.tile([128, CAP], f32, tag="ed")
                nc.sync.dma_start(ed_sb[:], ed_t[blk, :, :])
                gl = gi_p.tile([128, CAPL * 8], i16, tag="gil")
                nc.sync.dma_start(gl[:], gi_lo[blk, :, :])
                gh = gi_p.tile([128, CAPH * 8], i16, tag="gih")
                nc.sync.dma_start(gh[:], gi_hi[blk, :, :])

                xt = x_p.tile([128, CAP, D], bf16)
                nc.gpsimd.dma_gather(
                    xt[:, 0:CAPL, :], v_bf[0:HALF, :], gl[:],
                    num_idxs=CAPL * 128, num_idxs_reg=CAPL * 128,
                    elem_size=D, single_packet=False, queue_num=nextq())
                nc.gpsimd.dma_gather(
                    xt[:, CAPL:CAP, :], v_bf[HALF:N, :], gh[:],
                    num_idxs=CAPH * 128, num_idxs_reg=CAPH * 128,
                    elem_size=D, single_packet=False, queue_num=nextq())

                a_sb = a_p.tile([128, CAP, 128], bf16)
                for c in range(CAP):
                    eng = nc.vector
                    eng.tensor_scalar(
                        out=a_sb[:, c, :], in0=iota_r[:],
                        scalar1=er_sb[:, c:c + 1],
                        scalar2=ed_sb[:, c:c + 1],
                        op0=mybir.AluOpType.is_equal,
                        op1=mybir.AluOpType.mult)

                ps = ps_p.tile([128, D], f32)
                for c in range(CAP):
                    nc.tensor.matmul(
                        ps[:], lhsT=a_sb[:, c, :], rhs=xt[:, c, :],
                        start=(c == 0), stop=False)
                nc.tensor.matmul(ps[:], lhsT=ones_k1[:], rhs=mu_sb[:],
                                 start=False, stop=True)
                h_sb = h_p.tile([128, WW], bf16)
                nc.scalar.activation(
                    h_sb[:, 0:D], ps[:],
                    mybir.ActivationFunctionType.Sigmoid)
                if WW > D:
                    nc.vector.memset(h_sb[:, D:WW], 0.0)
                    nc.vector.memset(h_sb[:, D:D + 1], 1.0)
                nc.sync.dma_start(h_d[blk * 128:(blk + 1) * 128, :], h_sb[:])

            # ---------------- phase 2 ----------------
            calls = []
            for off in range(0, TLO * 128, GT * 128):
                calls.append((off, min(GT * 128, TLO * 128 - off), 0))
            for off in range(TLO * 128, TS, GT * 128):
                calls.append((off, min(GT * 128, TS - off), 1))

            for (off, sl, hseg) in calls:
                nt = sl // 128
                t0_ = off // 128
                hit = pi_p.tile([128, sl // 16], i16, tag="hit")
                nc.sync.dma_start(
                    hit[:], hi_t[:, off // 16:(off + sl) // 16])
                wit = pi_p.tile([128, sl // 16], i16, tag="wit")
                nc.sync.dma_start(
                    wit[:], wi_t[:, off // 16:(off + sl) // 16])

                ht = ht_p.tile([128, nt, WW], bf16)
                nc.gpsimd.dma_gather(
                    ht[:], h_d[:, :], hit[:], num_idxs=sl, num_idxs_reg=sl,
                    elem_size=WW, single_packet=False, queue_num=nextq())
                wt = wt_p.tile([128, nt, WW], bf16)
                wsrc = w_bf[0:HALF, :] if hseg == 0 else w_bf[HALF:N, :]
                nc.gpsimd.dma_gather(
                    wt[:], wsrc, wit[:], num_idxs=sl, num_idxs_reg=sl,
                    elem_size=WW, single_packet=False, queue_num=nextq())

                for t in range(nt):
                    gt = t0_ + t
                    prod = tr_p.tile([128, WW], bf16)
                    nc.vector.tensor_tensor(
                        out=prod[:], in0=ht[:, t, :], in1=wt[:, t, :],
                        op=mybir.AluOpType.mult)
                    if t % 2 == 0:
                        trash = tr_p.tile([128, WW], bf16, tag="trash")
                        nc.scalar.activation(
                            trash[:], prod[:],
                            mybir.ActivationFunctionType.Identity,
                            accum_out=sc_sb[:, gt:gt + 1])
                    else:
                        nc.vector.tensor_reduce(
                            out=sc_sb[:, gt:gt + 1], in_=prod[:],
                            axis=mybir.AxisListType.X,
                            op=mybir.AluOpType.add)

            nc.sync.dma_start(scores[:, :], sc_sb[:])

    nc.compile()
    return nc


def _gather_lanes(nc):
    """DMASW lane of each dma_gather, in emission order (None if absent)."""
    gs = []
    for blk in nc.m.functions[0].blocks:
        for inst in blk.instructions:
            if type(inst).__name__ != "InstDMAGatherAnt":
                continue
            lane = None
            si = inst.sync_info
            for u in (list(si.on_update) if si is not None else []):
                nm = getattr(u, "ant_name", None) or ""
                if nm.startswith("DMASW"):
                    lane = int(nm[5:].split("_")[0])
                    break
            gs.append((int(inst.name.split("-")[1]), lane))
    gs.sort()
    return [lane for _, lane in gs]


def _build_multiqueue(CAPL, CAPH, CL, CH, WW, dup=1):
    """Two-pass build: assign each gather queue = (its DMASW lane) % 4 so a
    Tile semaphore lane is only ever used by one SWDGE queue. Falls back to
    single-queue if the assignment doesn't reach a fixed point."""
    nc = _build_program(CAPL, CAPH, CL, CH, WW, dup=dup, queues=None)
    lanes = _gather_lanes(nc)
    if any(ln is None for ln in lanes):
        return nc
    queues = [ln % 4 for ln in lanes]
    for _ in range(3):
        nc2 = _build_program(CAPL, CAPH, CL, CH, WW, dup=dup,
                             queues=queues)
        lanes2 = _gather_lanes(nc2)
        if any(ln is None for ln in lanes2):
            return nc
        queues2 = [ln % 4 for ln in lanes2]
        if queues2 == queues:
            return nc2
        queues = queues2
    return nc


def _idx_table(idx2d):
    """[G, n] int16 -> [G, 128, n//16] wrapped-16/replicated-8 format."""
    G, n = idx2d.shape
    arr = idx2d.reshape(G, n // 16, 16).swapaxes(1, 2)  # [G, 16, n//16]
    return np.ascontiguousarray(np.tile(arr, (1, 8, 1)))


def kernel(idx, dat, m, n, i, j, v, mu, w, b):
    global LAST_RESULTS
    idx = np.asarray(idx)
    dat = np.asarray(dat, np.float32)
    i = np.asarray(i).astype(np.int64)
    j = np.asarray(j).astype(np.int64)
    v = np.asarray(v, np.float32)
    mu_np = np.asarray(mu, np.float32).reshape(1, D)
    w_np = np.asarray(w, np.float32)
    b_np = np.asarray(b, np.float32).reshape(-1)
    rows = idx[0].astype(np.int64)
    cols = idx[1].astype(np.int64)
    NNZ = rows.shape[0]
    P = i.shape[0]
    assert int(m) == M and int(n) == N
    assert v.shape == (N, D) and w_np.shape == (N, D)

    use_b = bool(np.any(b_np))
    WW = 384 if use_b else 256

    # ---------------- phase 1 host prep ----------------
    core_e = rows // RPC
    lrow = rows - core_e * RPC
    blk = lrow >> 7
    r_in_blk = (lrow & 127).astype(np.int64)
    half = (cols >= HALF).astype(np.int64)

    gkey = (core_e * BLOCKS + blk) * 2 + half
    order = np.lexsort((cols, gkey))
    gsorted = gkey[order]
    ngroups = NCORES * BLOCKS * 2
    counts = np.bincount(gsorted, minlength=ngroups)
    cnt_lo = counts[0::2]
    cnt_hi = counts[1::2]
    CAPL = max(1, int(np.ceil(cnt_lo.max() / 128)))
    CAPH = max(1, int(np.ceil(cnt_hi.max() / 128)))
    CAP = CAPL + CAPH
    gstart = np.zeros(ngroups + 1, np.int64)
    gstart[1:] = np.cumsum(counts)
    pos_in_group = np.arange(NNZ) - gstart[gsorted]
    eslot = pos_in_group + (gsorted % 2) * (CAPL * 128)
    g2 = gsorted // 2
    e_core = g2 // BLOCKS
    e_blk = g2 % BLOCKS
    echunk = eslot >> 7
    e_in_chunk = eslot & 127

    er_np = np.zeros((NCORES, BLOCKS, 128, CAP), np.float32)
    ed_np = np.zeros((NCORES, BLOCKS, 128, CAP), np.float32)
    er_np[e_core, e_blk, e_in_chunk, echunk] = r_in_blk[order]
    ed_np[e_core, e_blk, e_in_chunk, echunk] = dat[order]

    gi = np.zeros((NCORES, BLOCKS, CAP * 128), np.int16)
    gi[e_core, e_blk, eslot] = (cols[order] - half[order] * HALF).astype(
        np.int16)
    gi_lo = _idx_table(gi[:, :, :CAPL * 128].reshape(-1, CAPL * 128)).reshape(
        NCORES, BLOCKS, 128, CAPL * 8)
    gi_hi = _idx_table(gi[:, :, CAPL * 128:].reshape(-1, CAPH * 128)).reshape(
        NCORES, BLOCKS, 128, CAPH * 8)

    # ---------------- phase 2 host prep ----------------
    p_core = i // RPC
    il = (i - p_core * RPC).astype(np.int64)
    pblk = il >> 7
    r_il = (il & 127).astype(np.int64)
    jhalf = (j >= HALF).astype(np.int64)
    # group key: (core, jhalf, block); j-sorted inside each group
    pkey = (p_core * 2 + jhalf) * BLOCKS + pblk
    porder = np.lexsort((j, pkey))
    ksort = pkey[porder]
    npg = NCORES * 2 * BLOCKS
    pcounts = np.bincount(ksort, minlength=npg)
    pc3 = pcounts.reshape(NCORES, 2, BLOCKS)
    CL = [max(1, int(np.ceil(pc3[:, 0, bb].max() / 128)))
          for bb in range(BLOCKS)]
    CH = [max(1, int(np.ceil(pc3[:, 1, bb].max() / 128)))
          for bb in range(BLOCKS)]
    TT = sum(CL) + sum(CH)
    TS = TT * 128

    base = np.zeros((2, BLOCKS), np.int64)
    acc = 0
    for bb in range(BLOCKS):
        base[0, bb] = acc
        acc += CL[bb] * 128
    for bb in range(BLOCKS):
        base[1, bb] = acc
        acc += CH[bb] * 128
    assert acc == TS

    pstart = np.zeros(npg + 1, np.int64)
    pstart[1:] = np.cumsum(pcounts)
    pos2 = np.arange(P) - pstart[ksort]
    k_half = (ksort // BLOCKS) % 2
    k_blk = ksort % BLOCKS
    k_core = ksort // (2 * BLOCKS)
    slot = base[k_half, k_blk] + pos2

    ri_np = np.zeros((NCORES, TS), np.int16)
    ri_np[k_core, slot] = r_il[porder].astype(np.int16)
    wi_np = np.zeros((NCORES, TS), np.int16)
    wi_np[k_core, slot] = (j[porder] - k_half * HALF).astype(np.int16)
    slot_of_pair = np.empty(P, np.int64)
    slot_of_pair[porder] = k_core * TS + slot

    ri_t = _idx_table(ri_np)
    wi_t = _idx_table(wi_np)
    id_np = np.eye(128, dtype=BF16)

    # ---------------- build inputs ----------------
    v_bf = np.ascontiguousarray(v.astype(BF16))
    if use_b:
        w_aug = np.zeros((N, WW), np.float32)
        w_aug[:, :D] = w_np
        w_aug[:, D] = b_np
        w_bf = np.ascontiguousarray(w_aug.astype(BF16))
    else:
        w_bf = np.ascontiguousarray(w_np.astype(BF16))
    mu_bf = np.ascontiguousarray(mu_np.astype(BF16))

    key = (CAPL, CAPH, TLO, THI, WW)
    if _BUILD_CACHE.get("key") != key:
        _BUILD_CACHE.clear()
        _BUILD_CACHE["key"] = key
        _BUILD_CACHE["nc"] = _build_multiqueue(CAPL, CAPH, TLO, THI, WW)
    nc = _BUILD_CACHE["nc"]

    in_maps = []
    for c in range(NCORES):
        in_maps.append({
            "v_bf": v_bf,
            "w_bf": w_bf,
            "mu_bf": mu_bf,
            "er_t": er_np[c],
            "ed_t": ed_np[c],
            "gi_lo": gi_lo[c],
            "gi_hi": gi_hi[c],
            "ri_t": ri_t[c],
            "wi_t": wi_t[c],
            "id_t": id_np,
        })

    res = run_bass_kernel_spmd(
        nc, in_maps, core_ids=list(range(NCORES)),
        trace=bool(int(os.environ.get("KERNEL_TRACE", "0"))))
    LAST_RESULTS = res

    if os.environ.get("KERNEL_BENCH", "0") == "1":
        _benchmark(nc, in_maps)

    flat = np.concatenate(
        [np.asarray(res.results[c]["scores"], np.float32).T.reshape(-1)
         for c in range(NCORES)])
    return flat[slot_of_pair].astype(np.float32)


def _benchmark(nc, in_maps, iters=10):
    import time

    def timeit(run, n):
        for _ in range(2):
            run()
        ts = []
        for _ in range(n):
            t0 = time.perf_counter()
            run()
            ts.append(time.perf_counter() - t0)
        return np.array(ts)

    run1, _ = _make_bench(nc, in_maps, nrep=1)
    times = timeit(run1, iters)
    print(f"exec wall: min {times.min()*1e6:.0f} us  "
          f"median {np.median(times)*1e6:.0f} us  "
          f"mean {times.mean()*1e6:.0f} us")
    print(f"HW exec time: {times.min()*1e9:.0f} ns")
    nrep = int(os.environ.get("K_NREP", "0"))
    if nrep > 1:
        runN, _ = _make_bench(nc, in_maps, nrep=nrep)
        tN = timeit(runN, iters)
        body = (tN.min() - times.min()) / (nrep - 1)
        print(f"nrep={nrep} wall: min {tN.min()*1e6:.0f} us  "
              f"median {np.median(tN)*1e6:.0f} us")
        print(f"body estimate: {body*1e6:.1f} us per exec")
    return times


def _make_bench(nc, in_maps, nrep=None):
    """Build a timed executor: inputs pre-placed on device (mirrors
    bass2jax.run_bass_via_pjrt's multi-core path)."""
    import jax
    from jax.sharding import Mesh, NamedSharding, PartitionSpec

    from concourse import bass2jax
    from concourse.bass2jax import _bass_exec_p, install_neuronx_cc_hook

    install_neuronx_cc_hook()
    n_cores = NCORES
    part_name = (nc.partition_id_tensor.name
                 if nc.partition_id_tensor else None)
    in_names = []
    out_names = []
    out_avals = []
    zero_outs = []
    for alloc in nc.m.functions[0].allocations:
        if not isinstance(alloc, mybir.MemoryLocationSet):
            continue
        name = alloc.memorylocations[0].name
        if alloc.kind == "ExternalInput":
            if name != part_name:
                in_names.append(name)
        elif alloc.kind == "ExternalOutput":
            out_names.append(name)
            shape = tuple(alloc.tensor_shape)
            dtype = mybir.dt.np(alloc.dtype)
            out_avals.append(jax.core.ShapedArray(shape, dtype))
            zero_outs.append(np.zeros(shape, dtype))
    n_params = len(in_names)
    n_outs = len(out_avals)
    all_names = in_names + out_names
    if part_name is not None:
        all_names = all_names + [part_name]

    if nrep is None:
        nrep = 1

    def _body(*args):
        ins = list(args[:n_params])
        outs_all = []
        for r in range(nrep):
            operands = ins + list(
                args[n_params + r * n_outs:n_params + (r + 1) * n_outs])
            if part_name is not None:
                operands.append(bass2jax.partition_id_tensor())
            outs = _bass_exec_p.bind(
                *operands,
                out_avals=tuple(out_avals),
                in_names=tuple(all_names),
                out_names=tuple(out_names),
                lowering_input_output_aliases=(),
                sim_require_finite=True,
                sim_require_nnan=True,
                nc=nc,
            )
            outs_all.extend(outs)
        return tuple(outs_all)

    devices = jax.devices()[:n_cores]
    mesh = Mesh(np.asarray(devices), ("core",))
    shard_map = bass2jax.shard_map
    n_zeros = nrep * n_outs
    sharded = jax.jit(
        shard_map(_body, mesh=mesh,
                  in_specs=(PartitionSpec("core"),) * (n_params + n_zeros),
                  out_specs=(PartitionSpec("core"),) * n_zeros,
                  check_rep=False),
        keep_unused=True)

    sh = NamedSharding(mesh, PartitionSpec("core"))
    dev_in = [
        jax.device_put(
            np.concatenate([np.asarray(in_maps[c][nm]) for c in
                            range(n_cores)], axis=0), sh)
        for nm in in_names]
    dev_zeros = [
        jax.device_put(
            np.zeros((n_cores * z.shape[0], *z.shape[1:]), z.dtype), sh)
        for z in zero_outs] * nrep

    def run():
        outs = sharded(*dev_in, *dev_zeros)
        jax.block_until_ready(outs)
        return outs

    return run, nrep
